# revision 4
# baseline (speedup 1.0000x reference)
"""Bass/Trainium2 kernel for the (dead-attention) GAT reference.

Effective math (see reference):
    h1  = x @ W1f                 W1f = W1.transpose(1,0,2).reshape(256,128)
    hp1 = elu(adj @ h1)
    h2  = hp1 @ W2f               W2f = W2.transpose(1,0,2).reshape(128,128)
    hp2 = elu(adj @ h2)
    y   = elu(hp2 @ Wout + bout)
    out = log_softmax(y, axis=1)

Distribution: adj is sharded row-wise across 8 cores (2048 rows each).
Each core computes h for its own rows, an on-device AllGather replicates
h, then each core streams its adj shard (pre-transposed + fp16 on host)
from HBM through the PE array:  hpT[128 feat, 2048 rows] = sum_k
h[kblock].T-free @ adjT[kblock].  All accumulation is fp32 in PSUM; only
the two big streamed matmuls run in fp16 (max elementwise rel err vs
fp32 reference ~3e-4, dominated by rounding adj/h to fp16).
"""

import sys

import numpy as np

sys.path.insert(0, "/opt/trn_rl_repo")

N = 16384  # nodes
F = 256  # input features
D = 128  # hidden width (nheads*nhid)
C = 32  # classes
NCORES = 8
S = N // NCORES  # rows per core

_nc_cache = {}


def build_gat_nc(n_total=N, ncores=NCORES, enable_asserts=False):
    """Build the SPMD Bass program (one program, runs on all cores)."""
    from concourse import bacc, mybir, tile

    s = n_total // ncores  # shard rows per core
    kb = n_total // 128  # contraction blocks for the big matmul
    rc = s // 128  # 128-row chunks in this core's shard
    f32 = mybir.dt.float32
    f16 = mybir.dt.float16
    AF = mybir.ActivationFunctionType
    OP = mybir.AluOpType
    # n-chunks of the big-matmul output (<=512 fp32 per PSUM bank)
    nw = [min(512, s - i) for i in range(0, s, 512)]
    no = [i for i in range(0, s, 512)]

    nc = bacc.Bacc(
        "TRN2",
        target_bir_lowering=False,
        debug=False,
        enable_asserts=enable_asserts,
        num_devices=ncores,
    )

    adjt = nc.dram_tensor("adjt", [n_total, s], f16, kind="ExternalInput")
    xt = nc.dram_tensor("xt", [F, s], f32, kind="ExternalInput")
    w1 = nc.dram_tensor("w1", [F, D], f32, kind="ExternalInput")
    w2 = nc.dram_tensor("w2", [D, D], f32, kind="ExternalInput")
    wout = nc.dram_tensor("wout", [D, C], f32, kind="ExternalInput")
    bb = nc.dram_tensor("bb", [128, C], f32, kind="ExternalInput")
    out = nc.dram_tensor("out", [s, C], f32, kind="ExternalOutput")

    rg = [list(range(ncores))]

    with tile.TileContext(nc) as tc:
        with (
            tc.tile_pool(name="dram", bufs=1, space="DRAM") as dram,
            tc.tile_pool(name="const", bufs=1) as const,
            tc.tile_pool(name="hfull", bufs=1) as hpool,
            tc.tile_pool(name="adjs", bufs=4) as apool,
            tc.tile_pool(name="xe", bufs=2) as xepool,
            tc.tile_pool(name="hsb", bufs=3) as hsbpool,
            tc.tile_pool(name="tmp", bufs=2) as tmp,
            tc.tile_pool(name="outp", bufs=2) as outp,
            tc.tile_pool(name="stat", bufs=2) as stat,
            tc.tile_pool(name="psb", bufs=4, space="PSUM") as psb,
            tc.tile_pool(name="pss", bufs=2, space="PSUM") as pss,
            tc.tile_pool(name="psy", bufs=2, space="PSUM") as psy,
        ):
            # --- replicated constants / per-core x slice ---
            sx = const.tile([128, 2, s], f32, tag="sx")
            nc.sync.dma_start(sx[:], xt.ap().rearrange("(a p) m -> p a m", p=128))
            w1s = const.tile([128, 2, D], f32, tag="w1s")
            nc.sync.dma_start(w1s[:], w1.ap().rearrange("(a p) m -> p a m", p=128))
            w2s = const.tile([128, D], f32, tag="w2s")
            nc.sync.dma_start(w2s[:], w2.ap())
            wouts = const.tile([128, C], f32, tag="wouts")
            nc.sync.dma_start(wouts[:], wout.ap())
            bbs = const.tile([128, C], f32, tag="bbs")
            nc.sync.dma_start(bbs[:], bb.ap())

            # --- DRAM bounce buffers for the collectives ---
            h1b = dram.tile([s, D], f16, tag="h1b")
            h1f = dram.tile([n_total, D], f16, tag="h1f", addr_space="Shared")
            h2b = dram.tile([s, D], f16, tag="h2b")
            h2f = dram.tile([n_total, D], f16, tag="h2f", addr_space="Shared")

            def small_mm_store(lhs_tile, lhs_chunks, rhs_tile, bounce):
                # h[rowchunk c] = sum_a lhsT_a.T @ rhs_a, cast fp16, store
                br = bounce.rearrange("(c p) m -> c p m", p=128)
                for c in range(rc):
                    ph = pss.tile([128, D], f32, tag="pss")
                    cs = slice(c * 128, (c + 1) * 128)
                    na = len(lhs_chunks)
                    for a in lhs_chunks:
                        nc.tensor.matmul(
                            ph[:],
                            lhs_tile[:, a, cs] if na > 1 else lhs_tile[:, cs],
                            rhs_tile[:, a, :] if na > 1 else rhs_tile[:],
                            start=(a == 0),
                            stop=(a == na - 1),
                        )
                    hsb = hsbpool.tile([128, D], f16, tag="hsb")
                    nc.vector.tensor_copy(hsb[:], ph[:])
                    nc.sync.dma_start(br[c], hsb[:])

            def all_gather(src, dst):
                nc.gpsimd.collective_compute(
                    "AllGather",
                    OP.bypass,
                    ins=[src.opt()],
                    outs=[dst.opt()],
                    replica_groups=rg,
                )

            def big_layer(hf):
                # hpT[128 feat, s rows] += h[kblk].T-stationary @ adjT[kblk]
                hs = hpool.tile([128, kb, D], f16, tag="hfull")
                hr = hf.rearrange("(k p) m -> k p m", p=128)
                for k in range(kb):
                    nc.sync.dma_start(hs[:, k, :], hr[k])
                ps = [
                    psb.tile([128, w], f32, tag="big", name=f"pbig{i}")
                    for i, w in enumerate(nw)
                ]
                ar = adjt.ap().rearrange("(k p) m -> k p m", p=128)
                for k in range(kb):
                    at = apool.tile([128, s], f16, tag="adj")
                    nc.sync.dma_start(at[:], ar[k])
                    for n, (o, w) in enumerate(zip(no, nw)):
                        nc.tensor.matmul(
                            ps[n][:],
                            hs[:, k, :],
                            at[:, o : o + w],
                            start=(k == 0),
                            stop=(k == kb - 1),
                        )
                return ps

            def elu_chunks(ps, dst):
                # dst[:, s] = elu(ps chunks), fp32
                for n, (o, w) in enumerate(zip(no, nw)):
                    neg = tmp.tile([128, 512], f32, tag="neg")
                    nc.vector.tensor_scalar_min(neg[:, :w], ps[n][:], 0.0)
                    ex = tmp.tile([128, 512], f32, tag="ex")
                    nc.scalar.activation(ex[:, :w], neg[:, :w], AF.Exp)
                    pm1 = tmp.tile([128, 512], f32, tag="pm1")
                    nc.vector.tensor_scalar(
                        pm1[:, :w], ps[n][:], 0.0, -1.0, op0=OP.max, op1=OP.add
                    )
                    nc.vector.tensor_add(dst[:, o : o + w], ex[:, :w], pm1[:, :w])

            # ---- layer 1 ----
            small_mm_store(sx, [0, 1], w1s, h1b)
            all_gather(h1b, h1f)
            ps1 = big_layer(h1f)
            x2t = xepool.tile([128, s], f32, tag="xe")
            elu_chunks(ps1, x2t)

            # ---- layer 2 ----
            small_mm_store(x2t, [0], w2s, h2b)
            all_gather(h2b, h2f)
            ps2 = big_layer(h2f)
            x3t = xepool.tile([128, s], f32, tag="xe")
            elu_chunks(ps2, x3t)

            # ---- output layer + log_softmax (per 128-row chunk) ----
            outr = out.ap().rearrange("(c p) m -> c p m", p=128)
            for c in range(rc):
                py = psy.tile([128, C], f32, tag="psy")
                cs = slice(c * 128, (c + 1) * 128)
                nc.tensor.matmul(py[:], x3t[:, cs], wouts[:], start=True, stop=True)
                z = outp.tile([128, C], f32, tag="z")
                nc.vector.tensor_add(z[:], py[:], bbs[:])
                # elu
                negz = outp.tile([128, C], f32, tag="negz")
                nc.vector.tensor_scalar_min(negz[:], z[:], 0.0)
                ez = outp.tile([128, C], f32, tag="ez")
                nc.scalar.activation(ez[:], negz[:], AF.Exp)
                pmz = outp.tile([128, C], f32, tag="pmz")
                nc.vector.tensor_scalar(
                    pmz[:], z[:], 0.0, -1.0, op0=OP.max, op1=OP.add
                )
                zz = outp.tile([128, C], f32, tag="zz")
                nc.vector.tensor_add(zz[:], ez[:], pmz[:])
                # log_softmax along the class (free) axis
                negm = stat.tile([128, 1], f32, tag="negm")
                nc.vector.tensor_reduce(
                    negm[:], zz[:], axis=mybir.AxisListType.X, op=OP.max, negate=True
                )
                es = outp.tile([128, C], f32, tag="es")
                ssum = stat.tile([128, 1], f32, tag="ssum")
                nc.scalar.activation(
                    es[:], zz[:], AF.Exp, bias=negm[:], accum_out=ssum[:]
                )
                lse = stat.tile([128, 1], f32, tag="lse")
                nc.scalar.activation(lse[:], ssum[:], AF.Ln)
                osb = outp.tile([128, C], f32, tag="osb")
                nc.vector.tensor_scalar(
                    osb[:], zz[:], negm[:], lse[:], op0=OP.add, op1=OP.subtract
                )
                nc.sync.dma_start(outr[c], osb[:])

    nc.compile()
    return nc


def make_in_maps(x, adj, W1, W2, Wout, bout, ncores=NCORES):
    n_total = adj.shape[0]
    s = n_total // ncores
    f, d = W1.shape[1], W1.shape[0] * W1.shape[2]
    w1f = np.ascontiguousarray(
        W1.transpose(1, 0, 2).reshape(f, d).astype(np.float32)
    )
    w2f = np.ascontiguousarray(
        W2.transpose(1, 0, 2).reshape(d, d).astype(np.float32)
    )
    woutf = np.ascontiguousarray(Wout.astype(np.float32))
    bbf = np.ascontiguousarray(
        np.broadcast_to(bout.astype(np.float32), (128, Wout.shape[1]))
    )
    adj_t = adj.T  # view; [:, rows] below copies
    in_maps = []
    for c in range(ncores):
        rows = slice(c * s, (c + 1) * s)
        in_maps.append(
            {
                "adjt": adj_t[:, rows].astype(np.float16),
                "xt": np.ascontiguousarray(x[rows].T.astype(np.float32)),
                "w1": w1f,
                "w2": w2f,
                "wout": woutf,
                "bb": bbf,
            }
        )
    return in_maps


def kernel(x, adj, W1, W2, Wout, bout):
    from concourse import bass_utils

    x = np.asarray(x)
    adj = np.asarray(adj)
    in_maps = make_in_maps(x, adj, np.asarray(W1), np.asarray(W2),
                           np.asarray(Wout), np.asarray(bout))
    if "nc" not in _nc_cache:
        _nc_cache["nc"] = build_gat_nc()
    res = bass_utils.run_bass_kernel_spmd(
        _nc_cache["nc"], in_maps, core_ids=list(range(NCORES))
    )
    return np.concatenate([r["out"] for r in res.results], axis=0).astype(np.float32)


# revision 5
# speedup vs baseline: 1.1285x; 1.1285x over previous
"""Bass/Trainium2 kernel for the (dead-attention) GAT reference.

Effective math (see reference):
    h1  = x @ W1f                 W1f = W1.transpose(1,0,2).reshape(256,128)
    hp1 = elu(adj @ h1)
    h2  = hp1 @ W2f               W2f = W2.transpose(1,0,2).reshape(128,128)
    hp2 = elu(adj @ h2)
    y   = elu(hp2 @ Wout + bout)
    out = log_softmax(y, axis=1)

Distribution: adj is sharded row-wise across 8 cores (2048 rows each),
uploaded pre-transposed + fp16.  h1 is computed REPLICATED on every core
(x is tiny), so layer 1 needs no collective and the big adj stream can
start immediately.  h2 depends on the sharded layer-1 output, so one
fp16 AllGather replicates it between the layers.  Each core streams its
67 MB adj shard from HBM through the PE array twice:
    hpT[128 feat, 2048 rows] = sum_k h[kblk 128 rows].T-stationary @ adjT[kblk]
fp32 accumulation in PSUM; fp16 only on the two big streamed matmuls
(max elementwise rel err vs the fp32 reference ~2e-4).
"""

import sys

import numpy as np

sys.path.insert(0, "/opt/trn_rl_repo")

N = 16384  # nodes
F = 256  # input features
D = 128  # hidden width (nheads*nhid)
C = 32  # classes
NCORES = 8
S = N // NCORES  # rows per core

_nc_cache = {}


def build_gat_nc(n_total=N, ncores=NCORES, enable_asserts=False, adj_bufs=16):
    """Build the SPMD Bass program (one program, runs on all cores)."""
    from concourse import bacc, mybir, tile

    s = n_total // ncores  # shard rows per core
    kb = n_total // 128  # contraction blocks for the big matmul
    rc = s // 128  # 128-row chunks in this core's shard
    f32 = mybir.dt.float32
    f16 = mybir.dt.float16
    AF = mybir.ActivationFunctionType
    OP = mybir.AluOpType
    # n-chunks of the big-matmul output (<=512 fp32 per PSUM bank)
    nw = [min(512, s - i) for i in range(0, s, 512)]
    no = [i for i in range(0, s, 512)]

    nc = bacc.Bacc(
        "TRN2",
        target_bir_lowering=False,
        debug=False,
        enable_asserts=enable_asserts,
        num_devices=ncores,
    )

    adjt = nc.dram_tensor("adjt", [n_total, s], f16, kind="ExternalInput")
    xc = nc.dram_tensor("xc", [kb * F, 128], f16, kind="ExternalInput")
    w1 = nc.dram_tensor("w1", [F, D], f16, kind="ExternalInput")
    w2 = nc.dram_tensor("w2", [D, D], f32, kind="ExternalInput")
    wout = nc.dram_tensor("wout", [D, C], f32, kind="ExternalInput")
    bb = nc.dram_tensor("bb", [128, C], f32, kind="ExternalInput")
    out = nc.dram_tensor("out", [s, C], f32, kind="ExternalOutput")

    rg = [list(range(ncores))]

    with tile.TileContext(nc) as tc:
        with (
            tc.tile_pool(name="dram", bufs=1, space="DRAM") as dram,
            tc.tile_pool(name="const", bufs=1) as const,
            tc.tile_pool(name="hfull", bufs=1) as hpool,
            tc.tile_pool(name="adjs", bufs=adj_bufs) as apool,
            tc.tile_pool(name="xcp", bufs=4) as xcpool,
            tc.tile_pool(name="xe", bufs=2) as xepool,
            tc.tile_pool(name="hsb", bufs=3) as hsbpool,
            tc.tile_pool(name="tmp", bufs=2) as tmp,
            tc.tile_pool(name="outp", bufs=2) as outp,
            tc.tile_pool(name="stat", bufs=2) as stat,
            tc.tile_pool(name="psb", bufs=4, space="PSUM") as psb,
            tc.tile_pool(name="pss", bufs=2, space="PSUM") as pss,
            tc.tile_pool(name="psy", bufs=2, space="PSUM") as psy,
        ):
            # two HWDGE rings: big streams alternate, side loads use ringB
            ringA, ringB = nc.sync, nc.scalar

            # --- replicated constants ---
            w1s = const.tile([128, 2, D], f16, tag="w1s")
            ringB.dma_start(w1s[:], w1.ap().rearrange("(a p) m -> p a m", p=128))
            w2s = const.tile([128, D], f32, tag="w2s")
            ringB.dma_start(w2s[:], w2.ap())
            wouts = const.tile([128, C], f32, tag="wouts")
            ringB.dma_start(wouts[:], wout.ap())
            bbs = const.tile([128, C], f32, tag="bbs")
            ringB.dma_start(bbs[:], bb.ap())

            # --- DRAM bounce buffers for the collective ---
            h2b = dram.tile([s, D], f16, tag="h2b")
            h2f = dram.tile([n_total, D], f16, tag="h2f", addr_space="Shared")

            def big_layer(hs):
                # hpT[128 feat, s rows] += h[kblk].T-stationary @ adjT[kblk]
                ps = [
                    psb.tile([128, w], f32, tag="big", name=f"pbig{i}")
                    for i, w in enumerate(nw)
                ]
                ar = adjt.ap().rearrange("(k p) m -> k p m", p=128)
                for k in range(kb):
                    at = apool.tile([128, s], f16, tag="adj")
                    (ringA if k % 2 == 0 else ringB).dma_start(at[:], ar[k])
                    for n, (o, w) in enumerate(zip(no, nw)):
                        nc.tensor.matmul(
                            ps[n][:],
                            hs[:, k, :],
                            at[:, o : o + w],
                            start=(k == 0),
                            stop=(k == kb - 1),
                        )
                return ps

            def elu_chunks(ps, dst):
                # dst[:, s] = elu(ps chunks), fp32
                for n, (o, w) in enumerate(zip(no, nw)):
                    neg = tmp.tile([128, 512], f32, tag="neg")
                    nc.vector.tensor_scalar_min(neg[:, :w], ps[n][:], 0.0)
                    ex = tmp.tile([128, 512], f32, tag="ex")
                    nc.scalar.activation(ex[:, :w], neg[:, :w], AF.Exp)
                    pm1 = tmp.tile([128, 512], f32, tag="pm1")
                    nc.vector.tensor_scalar(
                        pm1[:, :w], ps[n][:], 0.0, -1.0, op0=OP.max, op1=OP.add
                    )
                    nc.vector.tensor_add(dst[:, o : o + w], ex[:, :w], pm1[:, :w])

            # ---- layer 1: h1 replicated (no collective) ----
            # xc[k] holds x[k*128:(k+1)*128, :].T pre-chunked: [a, p, m] with
            # lhsT_a[p, m] = x[k*128 + m, a*128 + p]
            hs1 = hpool.tile([128, kb, D], f16, tag="hfull")
            xr = xc.ap().rearrange("(k a p) m -> k p a m", a=2, p=128)
            for k in range(kb):
                xck = xcpool.tile([128, 2, 128], f16, tag="xc")
                ringB.dma_start(xck[:], xr[k])
                ph = pss.tile([128, D], f32, tag="pss", name=f"ph1_{k}")
                nc.tensor.matmul(
                    ph[:], xck[:, 0, :], w1s[:, 0, :], start=True, stop=False
                )
                nc.tensor.matmul(
                    ph[:], xck[:, 1, :], w1s[:, 1, :], start=False, stop=True
                )
                nc.vector.tensor_copy(hs1[:, k, :], ph[:])
            ps1 = big_layer(hs1)
            x2t = xepool.tile([128, s], f32, tag="xe")
            elu_chunks(ps1, x2t)

            # ---- layer 2: h2 for own shard, AllGather, stream adj again ----
            h2br = h2b.rearrange("(c p) m -> c p m", p=128)
            for c in range(rc):
                ph2 = pss.tile([128, D], f32, tag="pss", name=f"ph2_{c}")
                cs = slice(c * 128, (c + 1) * 128)
                nc.tensor.matmul(ph2[:], x2t[:, cs], w2s[:], start=True, stop=True)
                hsb = hsbpool.tile([128, D], f16, tag="hsb")
                nc.vector.tensor_copy(hsb[:], ph2[:])
                ringA.dma_start(h2br[c], hsb[:])
            nc.gpsimd.collective_compute(
                "AllGather",
                OP.bypass,
                ins=[h2b.opt()],
                outs=[h2f.opt()],
                replica_groups=rg,
            )
            hs2 = hpool.tile([128, kb, D], f16, tag="hfull")
            h2fr = h2f.rearrange("(k p) m -> k p m", p=128)
            for k in range(kb):
                ringB.dma_start(hs2[:, k, :], h2fr[k])
            ps2 = big_layer(hs2)
            x3t = xepool.tile([128, s], f32, tag="xe")
            elu_chunks(ps2, x3t)

            # ---- output layer + log_softmax (per 128-row chunk) ----
            outr = out.ap().rearrange("(c p) m -> c p m", p=128)
            for c in range(rc):
                py = psy.tile([128, C], f32, tag="psy")
                cs = slice(c * 128, (c + 1) * 128)
                nc.tensor.matmul(py[:], x3t[:, cs], wouts[:], start=True, stop=True)
                z = outp.tile([128, C], f32, tag="z")
                nc.vector.tensor_add(z[:], py[:], bbs[:])
                # elu
                negz = outp.tile([128, C], f32, tag="negz")
                nc.vector.tensor_scalar_min(negz[:], z[:], 0.0)
                ez = outp.tile([128, C], f32, tag="ez")
                nc.scalar.activation(ez[:], negz[:], AF.Exp)
                pmz = outp.tile([128, C], f32, tag="pmz")
                nc.vector.tensor_scalar(
                    pmz[:], z[:], 0.0, -1.0, op0=OP.max, op1=OP.add
                )
                zz = outp.tile([128, C], f32, tag="zz")
                nc.vector.tensor_add(zz[:], ez[:], pmz[:])
                # log_softmax along the class (free) axis
                negm = stat.tile([128, 1], f32, tag="negm")
                nc.vector.tensor_reduce(
                    negm[:], zz[:], axis=mybir.AxisListType.X, op=OP.max, negate=True
                )
                es = outp.tile([128, C], f32, tag="es")
                ssum = stat.tile([128, 1], f32, tag="ssum")
                nc.scalar.activation(
                    es[:], zz[:], AF.Exp, bias=negm[:], accum_out=ssum[:]
                )
                lse = stat.tile([128, 1], f32, tag="lse")
                nc.scalar.activation(lse[:], ssum[:], AF.Ln)
                osb = outp.tile([128, C], f32, tag="osb")
                nc.vector.tensor_scalar(
                    osb[:], zz[:], negm[:], lse[:], op0=OP.add, op1=OP.subtract
                )
                ringA.dma_start(outr[c], osb[:])

    nc.compile()
    return nc


def make_in_maps(x, adj, W1, W2, Wout, bout, ncores=NCORES):
    n_total = adj.shape[0]
    s = n_total // ncores
    kb = n_total // 128
    f, d = W1.shape[1], W1.shape[0] * W1.shape[2]
    w1f = np.ascontiguousarray(
        W1.transpose(1, 0, 2).reshape(f, d).astype(np.float16)
    )
    w2f = np.ascontiguousarray(
        W2.transpose(1, 0, 2).reshape(d, d).astype(np.float32)
    )
    woutf = np.ascontiguousarray(Wout.astype(np.float32))
    bbf = np.ascontiguousarray(
        np.broadcast_to(bout.astype(np.float32), (128, Wout.shape[1]))
    )
    # xc[k, a, p, m] = x.T[a*128 + p, k*128 + m], flattened to [kb*F, 128]
    xt = x.T.astype(np.float16)  # [F, n_total]
    xcf = np.ascontiguousarray(
        xt.reshape(2, 128, kb, 128).transpose(2, 0, 1, 3).reshape(kb * f, 128)
    )
    adj_t = adj.T  # view; [:, rows] below copies
    in_maps = []
    for c in range(ncores):
        rows = slice(c * s, (c + 1) * s)
        in_maps.append(
            {
                "adjt": adj_t[:, rows].astype(np.float16),
                "xc": xcf,
                "w1": w1f,
                "w2": w2f,
                "wout": woutf,
                "bb": bbf,
            }
        )
    return in_maps


def kernel(x, adj, W1, W2, Wout, bout):
    from concourse import bass_utils

    x = np.asarray(x)
    adj = np.asarray(adj)
    in_maps = make_in_maps(x, adj, np.asarray(W1), np.asarray(W2),
                           np.asarray(Wout), np.asarray(bout))
    if "nc" not in _nc_cache:
        _nc_cache["nc"] = build_gat_nc()
    res = bass_utils.run_bass_kernel_spmd(
        _nc_cache["nc"], in_maps, core_ids=list(range(NCORES))
    )
    return np.concatenate([r["out"] for r in res.results], axis=0).astype(np.float32)


# revision 15
# speedup vs baseline: 1.3256x; 1.1746x over previous
"""Bass/Trainium2 kernel for the (dead-attention) GAT reference.

Effective math (see reference):
    h1  = x @ W1f                 W1f = W1.transpose(1,0,2).reshape(256,128)
    hp1 = elu(adj @ h1)
    h2  = hp1 @ W2f               W2f = W2.transpose(1,0,2).reshape(128,128)
    hp2 = elu(adj @ h2)
    y   = elu(hp2 @ Wout + bout)
    out = log_softmax(y, axis=1)

Distribution: adj is sharded row-wise across 8 cores (2048 rows each),
uploaded pre-transposed + fp16.  h1 is computed REPLICATED on every core
(x is tiny), so layer 1 needs no collective and the big adj stream can
start immediately.  h2 depends on the sharded layer-1 output, so one
fp16 AllGather replicates it between the layers.  Each core streams its
67 MB adj shard from HBM through the PE array twice:
    hpT[128 feat, 2048 rows] = sum_k h[kblk 128 rows].T-stationary @ adjT[kblk]
fp32 accumulation in PSUM; fp16 only on the two big streamed matmuls
(max elementwise rel err vs the fp32 reference ~2e-4).
"""

import sys

import numpy as np

sys.path.insert(0, "/opt/trn_rl_repo")

N = 16384  # nodes
F = 256  # input features
D = 128  # hidden width (nheads*nhid)
C = 32  # classes
NCORES = 8
S = N // NCORES  # rows per core

_nc_cache = {}


def build_gat_nc(n_total=N, ncores=NCORES, enable_asserts=False, adj_bufs=10):
    """Build the SPMD Bass program (one program, runs on all cores)."""
    from concourse import bacc, mybir, tile

    s = n_total // ncores  # shard rows per core
    kb = n_total // 128  # contraction blocks for the big matmul
    kg = 2  # k-blocks per adj DMA (1 MiB transfers)
    rc = s // 128  # 128-row chunks in this core's shard
    f32 = mybir.dt.float32
    f16 = mybir.dt.float16
    AF = mybir.ActivationFunctionType
    OP = mybir.AluOpType
    # n-chunks of the big-matmul output (<=512 fp32 per PSUM bank)
    nw = [min(512, s - i) for i in range(0, s, 512)]
    no = [i for i in range(0, s, 512)]

    nc = bacc.Bacc(
        "TRN2",
        target_bir_lowering=False,
        debug=False,
        enable_asserts=enable_asserts,
        num_devices=ncores,
    )

    adjt = nc.dram_tensor("adjt", [n_total, s], f16, kind="ExternalInput")
    xc = nc.dram_tensor("xc", [kb * 128, F], f16, kind="ExternalInput")
    w1 = nc.dram_tensor("w1", [F, D], f16, kind="ExternalInput")
    w2 = nc.dram_tensor("w2", [D, D], f32, kind="ExternalInput")
    wout = nc.dram_tensor("wout", [D, C], f32, kind="ExternalInput")
    bb = nc.dram_tensor("bb", [128, C], f32, kind="ExternalInput")
    out = nc.dram_tensor("out", [s, C], f32, kind="ExternalOutput")

    rg = [list(range(ncores))]

    with tile.TileContext(nc) as tc:
        with (
            tc.tile_pool(name="dram", bufs=1, space="DRAM") as dram,
            tc.tile_pool(name="const", bufs=1) as const,
            tc.tile_pool(name="hfull", bufs=1) as hpool,
            tc.tile_pool(name="adjs", bufs=adj_bufs) as apool,
            tc.tile_pool(name="xcp", bufs=4) as xcpool,
            tc.tile_pool(name="xe", bufs=2) as xepool,
            tc.tile_pool(name="hsb", bufs=3) as hsbpool,
            tc.tile_pool(name="tmp", bufs=2) as tmp,
            tc.tile_pool(name="outp", bufs=2) as outp,
            tc.tile_pool(name="stat", bufs=2) as stat,
            tc.tile_pool(name="psb", bufs=4, space="PSUM") as psb,
            tc.tile_pool(name="pss", bufs=2, space="PSUM") as pss,
            tc.tile_pool(name="psy", bufs=2, space="PSUM") as psy,
        ):
            # two HWDGE rings (sync/scalar) alternate the big adj stream;
            # small latency-critical transfers go to the SWDGE path (gpsimd)
            # so they don't queue behind megabyte prefetches
            ringA, ringB, ringC = nc.sync, nc.scalar, nc.gpsimd

            # --- replicated constants ---
            w1s = const.tile([128, 2, D], f16, tag="w1s")
            ringB.dma_start(w1s[:], w1.ap().rearrange("(a p) m -> p a m", p=128))
            w2s = const.tile([128, D], f32, tag="w2s")
            ringB.dma_start(w2s[:], w2.ap())
            wouts = const.tile([128, C], f32, tag="wouts")
            ringB.dma_start(wouts[:], wout.ap())
            bbs = const.tile([128, C], f32, tag="bbs")
            ringB.dma_start(bbs[:], bb.ap())

            # --- DRAM bounce buffers for the collective ---
            h2b = dram.tile([s, D], f16, tag="h2b")
            h2f = dram.tile([n_total, D], f16, tag="h2f", addr_space="Shared")

            def big_layer(hs):
                # hpT[128 feat, s rows] += h[kblk].T-stationary @ adjT[kblk]
                ps = [
                    psb.tile([128, w], f32, tag="big", name=f"pbig{i}")
                    for i, w in enumerate(nw)
                ]
                ar = adjt.ap().rearrange("(g j p) m -> g p j m", j=kg, p=128)
                for g in range(kb // kg):
                    at = apool.tile([128, kg, s], f16, tag="adj")
                    (ringA if g % 2 == 0 else ringB).dma_start(at[:], ar[g])
                    for j in range(kg):
                        k = g * kg + j
                        for n, (o, w) in enumerate(zip(no, nw)):
                            nc.tensor.matmul(
                                ps[n][:],
                                hs[:, k, :],
                                at[:, j, o : o + w],
                                start=(k == 0),
                                stop=(k == kb - 1),
                            )
                return ps

            def elu_chunks(ps, dst):
                # dst[:, s] = elu(ps chunks), fp32
                for n, (o, w) in enumerate(zip(no, nw)):
                    neg = tmp.tile([128, 512], f32, tag="neg")
                    nc.vector.tensor_scalar_min(neg[:, :w], ps[n][:], 0.0)
                    ex = tmp.tile([128, 512], f32, tag="ex")
                    nc.scalar.activation(ex[:, :w], neg[:, :w], AF.Exp)
                    pm1 = tmp.tile([128, 512], f32, tag="pm1")
                    nc.vector.tensor_scalar(
                        pm1[:, :w], ps[n][:], 0.0, -1.0, op0=OP.max, op1=OP.add
                    )
                    nc.vector.tensor_add(dst[:, o : o + w], ex[:, :w], pm1[:, :w])

            # ---- layer 1: h1 replicated (no collective) ----
            # xc[k*128 + p, a*128 + m] = x.T[a*128 + p, k*128 + m]:
            # per-partition-contiguous pre-chunked x.T
            hs1 = hpool.tile([128, kb, D], f16, tag="hfull")
            xr = xc.ap().rearrange("(k p) q -> k p q", p=128)
            for k in range(kb):
                xck = xcpool.tile([128, F], f16, tag="xc")
                ringB.dma_start(xck[:], xr[k])
                ph = pss.tile([128, D], f32, tag="pss", name=f"ph1_{k}")
                nc.tensor.matmul(
                    ph[:], xck[:, 0:128], w1s[:, 0, :], start=True, stop=False
                )
                nc.tensor.matmul(
                    ph[:], xck[:, 128:256], w1s[:, 1, :], start=False, stop=True
                )
                nc.vector.tensor_copy(hs1[:, k, :], ph[:])
            ps1 = big_layer(hs1)
            x2t = xepool.tile([128, s], f32, tag="xe")
            elu_chunks(ps1, x2t)

            # ---- layer 2: h2 for own shard, AllGather, stream adj again ----
            h2br = h2b.rearrange("(c p) m -> c p m", p=128)
            for c in range(rc):
                ph2 = pss.tile([128, D], f32, tag="pss", name=f"ph2_{c}")
                cs = slice(c * 128, (c + 1) * 128)
                nc.tensor.matmul(ph2[:], x2t[:, cs], w2s[:], start=True, stop=True)
                hsb = hsbpool.tile([128, D], f16, tag="hsb")
                nc.vector.tensor_copy(hsb[:], ph2[:])
                ringC.dma_start(h2br[c], hsb[:])
            nc.gpsimd.collective_compute(
                "AllGather",
                OP.bypass,
                ins=[h2b.opt()],
                outs=[h2f.opt()],
                replica_groups=rg,
            )
            hs2 = hpool.tile([128, kb, D], f16, tag="hfull")
            h2fr = h2f.rearrange("(g j p) m -> g p j m", j=4, p=128)
            hs2v = hs2.rearrange("p (g j) m -> p g j m", j=4)
            for g in range(kb // 4):
                ringC.dma_start(hs2v[:, g], h2fr[g])
            ps2 = big_layer(hs2)
            x3t = xepool.tile([128, s], f32, tag="xe")
            elu_chunks(ps2, x3t)

            # ---- output layer + log_softmax (per 128-row chunk) ----
            outr = out.ap().rearrange("(c p) m -> c p m", p=128)
            for c in range(rc):
                py = psy.tile([128, C], f32, tag="psy")
                cs = slice(c * 128, (c + 1) * 128)
                nc.tensor.matmul(py[:], x3t[:, cs], wouts[:], start=True, stop=True)
                z = outp.tile([128, C], f32, tag="z")
                nc.vector.tensor_add(z[:], py[:], bbs[:])
                # elu
                negz = outp.tile([128, C], f32, tag="negz")
                nc.vector.tensor_scalar_min(negz[:], z[:], 0.0)
                ez = outp.tile([128, C], f32, tag="ez")
                nc.scalar.activation(ez[:], negz[:], AF.Exp)
                pmz = outp.tile([128, C], f32, tag="pmz")
                nc.vector.tensor_scalar(
                    pmz[:], z[:], 0.0, -1.0, op0=OP.max, op1=OP.add
                )
                zz = outp.tile([128, C], f32, tag="zz")
                nc.vector.tensor_add(zz[:], ez[:], pmz[:])
                # log_softmax along the class (free) axis
                negm = stat.tile([128, 1], f32, tag="negm")
                nc.vector.tensor_reduce(
                    negm[:], zz[:], axis=mybir.AxisListType.X, op=OP.max, negate=True
                )
                es = outp.tile([128, C], f32, tag="es")
                ssum = stat.tile([128, 1], f32, tag="ssum")
                nc.scalar.activation(
                    es[:], zz[:], AF.Exp, bias=negm[:], accum_out=ssum[:]
                )
                lse = stat.tile([128, 1], f32, tag="lse")
                nc.scalar.activation(lse[:], ssum[:], AF.Ln)
                osb = outp.tile([128, C], f32, tag="osb")
                nc.vector.tensor_scalar(
                    osb[:], zz[:], negm[:], lse[:], op0=OP.add, op1=OP.subtract
                )
                ringC.dma_start(outr[c], osb[:])

    nc.compile()
    return nc


def make_in_maps(x, adj, W1, W2, Wout, bout, ncores=NCORES):
    n_total = adj.shape[0]
    s = n_total // ncores
    kb = n_total // 128
    f, d = W1.shape[1], W1.shape[0] * W1.shape[2]
    w1f = np.ascontiguousarray(
        W1.transpose(1, 0, 2).reshape(f, d).astype(np.float16)
    )
    w2f = np.ascontiguousarray(
        W2.transpose(1, 0, 2).reshape(d, d).astype(np.float32)
    )
    woutf = np.ascontiguousarray(Wout.astype(np.float32))
    bbf = np.ascontiguousarray(
        np.broadcast_to(bout.astype(np.float32), (128, Wout.shape[1]))
    )
    # xc[k*128 + p, a*128 + m] = x.T[a*128 + p, k*128 + m]
    xt = x.T.astype(np.float16)  # [F, n_total]
    xcf = np.ascontiguousarray(
        xt.reshape(2, 128, kb, 128).transpose(2, 1, 0, 3).reshape(kb * 128, f)
    )
    adj_t = adj.T  # view; [:, rows] below copies
    in_maps = []
    for c in range(ncores):
        rows = slice(c * s, (c + 1) * s)
        in_maps.append(
            {
                "adjt": adj_t[:, rows].astype(np.float16),
                "xc": xcf,
                "w1": w1f,
                "w2": w2f,
                "wout": woutf,
                "bb": bbf,
            }
        )
    return in_maps


def kernel(x, adj, W1, W2, Wout, bout):
    from concourse import bass_utils

    x = np.asarray(x)
    adj = np.asarray(adj)
    in_maps = make_in_maps(x, adj, np.asarray(W1), np.asarray(W2),
                           np.asarray(Wout), np.asarray(bout))
    if "nc" not in _nc_cache:
        _nc_cache["nc"] = build_gat_nc()
    res = bass_utils.run_bass_kernel_spmd(
        _nc_cache["nc"], in_maps, core_ids=list(range(NCORES))
    )
    return np.concatenate([r["out"] for r in res.results], axis=0).astype(np.float32)


# revision 18
# speedup vs baseline: 1.4423x; 1.0881x over previous
"""Bass/Trainium2 kernel for the (dead-attention) GAT reference.

Effective math (see reference):
    h1  = x @ W1f                 W1f = W1.transpose(1,0,2).reshape(256,128)
    hp1 = elu(adj @ h1)
    h2  = hp1 @ W2f               W2f = W2.transpose(1,0,2).reshape(128,128)
    hp2 = elu(adj @ h2)
    y   = elu(hp2 @ Wout + bout)
    out = log_softmax(y, axis=1)

Distribution: adj is sharded row-wise across 8 cores (2048 rows each),
uploaded pre-transposed + fp16.  h1 is computed REPLICATED on every core
(x is tiny), so layer 1 needs no collective and the big adj stream can
start immediately.  h2 is exchanged with one fp16 AllGather in
feature-major layout (contiguous 512 KB blocks both ways); the gathered
blocks are transposed back to node-major lhsT tiles on the PE.  Each
core streams its 67 MB adj shard from HBM through the PE array twice:
    hpT[128 feat, 2048 rows] = sum_k h[kblk 128 rows].T-stationary @ adjT[kblk]
fp32 accumulation in PSUM; fp16 on the streamed matmuls (max elementwise
rel err vs the fp32 reference ~6e-4).
"""

import sys

import numpy as np

sys.path.insert(0, "/opt/trn_rl_repo")

N = 16384  # nodes
F = 256  # input features
D = 128  # hidden width (nheads*nhid)
C = 32  # classes
NCORES = 8
S = N // NCORES  # rows per core

_nc_cache = {}


def build_gat_nc(n_total=N, ncores=NCORES, enable_asserts=False, adj_bufs=6, kg=4):
    """Build the SPMD Bass program (one program, runs on all cores)."""
    from concourse import bacc, masks, mybir, tile

    s = n_total // ncores  # shard rows per core
    kb = n_total // 128  # contraction blocks for the big matmul
    kb8 = kb // 8  # x chunk groups
    rc = s // 128  # 128-row chunks in this core's shard
    f32 = mybir.dt.float32
    f16 = mybir.dt.float16
    AF = mybir.ActivationFunctionType
    OP = mybir.AluOpType
    # n-chunks of the big-matmul output (<=512 fp32 per PSUM bank)
    nw = [min(512, s - i) for i in range(0, s, 512)]
    no = [i for i in range(0, s, 512)]

    nc = bacc.Bacc(
        "TRN2",
        target_bir_lowering=False,
        debug=False,
        enable_asserts=enable_asserts,
        num_devices=ncores,
    )

    adjt = nc.dram_tensor("adjt", [n_total, s], f16, kind="ExternalInput")
    xc = nc.dram_tensor("xc", [kb8 * 128, 8 * F], f16, kind="ExternalInput")
    w1 = nc.dram_tensor("w1", [F, D], f16, kind="ExternalInput")
    w2 = nc.dram_tensor("w2", [D, D], f16, kind="ExternalInput")
    wout = nc.dram_tensor("wout", [D, C], f32, kind="ExternalInput")
    bb = nc.dram_tensor("bb", [128, C], f32, kind="ExternalInput")
    out = nc.dram_tensor("out", [s, C], f32, kind="ExternalOutput")

    rg = [list(range(ncores))]

    with tile.TileContext(nc) as tc:
        with (
            tc.tile_pool(name="dram", bufs=1, space="DRAM") as dram,
            tc.tile_pool(name="const", bufs=1) as const,
            tc.tile_pool(name="hfull", bufs=1) as hpool,
            tc.tile_pool(name="adjs", bufs=adj_bufs) as apool,
            tc.tile_pool(name="hblkp", bufs=4) as hblkp,
            tc.tile_pool(name="xcp", bufs=2) as xcpool,
            tc.tile_pool(name="xe", bufs=2) as xepool,
            tc.tile_pool(name="hsb", bufs=2) as hsbpool,
            tc.tile_pool(name="tmp", bufs=1) as tmp,
            tc.tile_pool(name="outp", bufs=2) as outp,
            tc.tile_pool(name="stat", bufs=1) as stat,
            tc.tile_pool(name="psb", bufs=4, space="PSUM") as psb,
            tc.tile_pool(name="pss", bufs=2, space="PSUM") as pss,
            tc.tile_pool(name="psy", bufs=2, space="PSUM") as psy,
        ):
            # two HWDGE rings (sync/scalar) alternate the big adj stream;
            # tiny stores go to the SWDGE path (gpsimd)
            ringA, ringB, ringC = nc.sync, nc.scalar, nc.gpsimd

            # --- replicated constants ---
            w1s = const.tile([128, 2, D], f16, tag="w1s")
            ringB.dma_start(w1s[:], w1.ap().rearrange("(a p) m -> p a m", p=128))
            w2s = const.tile([128, D], f16, tag="w2s")
            ringB.dma_start(w2s[:], w2.ap())
            wouts = const.tile([128, C], f32, tag="wouts")
            ringB.dma_start(wouts[:], wout.ap())
            bbs = const.tile([128, C], f32, tag="bbs")
            ringB.dma_start(bbs[:], bb.ap())
            ident = const.tile([128, 128], f16, tag="ident")
            masks.make_identity(nc, ident[:])

            # --- DRAM bounce buffers for the collective (feature-major) ---
            h2b = dram.tile([128, s], f16, tag="h2b")
            h2f = dram.tile([128 * ncores, s], f16, tag="h2f", addr_space="Shared")

            def big_layer(hs):
                # hpT[128 feat, s rows] += h[kblk].T-stationary @ adjT[kblk]
                ps = [
                    psb.tile([128, w], f32, tag="big", name=f"pbig{i}")
                    for i, w in enumerate(nw)
                ]
                ar = adjt.ap().rearrange("(g j p) m -> g p j m", j=kg, p=128)
                for g in range(kb // kg):
                    at = apool.tile([128, kg, s], f16, tag="adj")
                    (ringA if g % 2 == 0 else ringB).dma_start(at[:], ar[g])
                    for j in range(kg):
                        k = g * kg + j
                        for n, (o, w) in enumerate(zip(no, nw)):
                            nc.tensor.matmul(
                                ps[n][:],
                                hs[:, k, :],
                                at[:, j, o : o + w],
                                start=(k == 0),
                                stop=(k == kb - 1),
                            )
                return ps

            def elu_chunks(ps, dst):
                # dst[:, s] = elu(ps chunks), fp32
                for n, (o, w) in enumerate(zip(no, nw)):
                    neg = tmp.tile([128, 512], f32, tag="neg")
                    nc.vector.tensor_scalar_min(neg[:, :w], ps[n][:], 0.0)
                    ex = tmp.tile([128, 512], f32, tag="ex")
                    nc.scalar.activation(ex[:, :w], neg[:, :w], AF.Exp)
                    pm1 = tmp.tile([128, 512], f32, tag="pm1")
                    nc.vector.tensor_scalar(
                        pm1[:, :w], ps[n][:], 0.0, -1.0, op0=OP.max, op1=OP.add
                    )
                    nc.vector.tensor_add(dst[:, o : o + w], ex[:, :w], pm1[:, :w])

            # ---- layer 1: h1 replicated (no collective) ----
            # xc group g holds 8 chunk-lhsTs contiguous per partition:
            # xc[g*128+p, ((j*2+a)*128)+m] = x.T[a*128+p, (g*8+j)*128+m]
            hs1 = hpool.tile([128, kb, D], f16, tag="hfull")
            xr = xc.ap().rearrange("(g p) q -> g p q", p=128)
            xg = None
            for k in range(kb):
                g, j = divmod(k, 8)
                if j == 0:
                    xg = xcpool.tile([128, 8, 2, 128], f16, tag="xg")
                    ringB.dma_start(
                        xg.rearrange("p j a m -> p (j a m)"), xr[g]
                    )
                ph = pss.tile([128, D], f32, tag="pss", name=f"ph1_{k}")
                nc.tensor.matmul(
                    ph[:], xg[:, j, 0, :], w1s[:, 0, :], start=True, stop=False
                )
                nc.tensor.matmul(
                    ph[:], xg[:, j, 1, :], w1s[:, 1, :], start=False, stop=True
                )
                nc.vector.tensor_copy(hs1[:, k, :], ph[:])
            ps1 = big_layer(hs1)
            x2t = xepool.tile([128, s], f32, tag="xe")
            elu_chunks(ps1, x2t)

            # ---- layer 2: h2 (feature-major), AllGather, transpose back ----
            x2h = xepool.tile([128, s], f16, tag="xeh")
            nc.vector.tensor_copy(x2h[:], x2t[:])
            h2sT = xepool.tile([128, s], f16, tag="h2sT")
            for c in range(rc):
                cs = slice(c * 128, (c + 1) * 128)
                ph2 = pss.tile([128, D], f32, tag="pss", name=f"ph2_{c}")
                # feat-major h2 chunk: W2f.T-stationary @ x2[feat, nodes]
                nc.tensor.matmul(ph2[:], w2s[:], x2h[:, cs], start=True, stop=True)
                nc.vector.tensor_copy(h2sT[:, cs], ph2[:])
            ringC.dma_start(h2b[:], h2sT[:])
            nc.gpsimd.collective_compute(
                "AllGather",
                OP.bypass,
                ins=[h2b.opt()],
                outs=[h2f.opt()],
                replica_groups=rg,
            )
            # gather blocks (contiguous), transpose on PE to node-major lhsT
            hs2 = hpool.tile([128, kb, D], f16, tag="hfull")
            hblks = []
            for r in range(ncores):
                hb = hblkp.tile([128, s], f16, tag="hblk", name=f"hblk{r}")
                # NOTE: keep these off the sync ring — SP-engine DMAs touching
                # collective-output buffers can hang (see
                # test_sync_dma_collective_hang)
                ringB.dma_start(hb[:], h2f[r * 128 : (r + 1) * 128, :])
                hblks.append(hb)
            for k in range(kb):
                r, jj = divmod(k, rc)
                pt = pss.tile([128, D], f16, tag="pss", name=f"pt_{k}")
                nc.tensor.transpose(
                    pt[:], hblks[r][:, jj * 128 : (jj + 1) * 128], ident[:]
                )
                nc.vector.tensor_copy(hs2[:, k, :], pt[:])
            ps2 = big_layer(hs2)
            x3t = xepool.tile([128, s], f32, tag="xe")
            elu_chunks(ps2, x3t)

            # ---- output layer + log_softmax ----
            outr = out.ap().rearrange("(c p) m -> c p m", p=128)
            zbig = outp.tile([128, rc, C], f32, tag="zbig", bufs=1)
            for c in range(rc):
                py = psy.tile([128, C], f32, tag="psy")
                cs = slice(c * 128, (c + 1) * 128)
                nc.tensor.matmul(py[:], x3t[:, cs], wouts[:], start=True, stop=True)
                nc.vector.tensor_add(zbig[:, c, :], py[:], bbs[:])
            # batched elu over [128, rc*C]
            zf = zbig.rearrange("p c m -> p (c m)")
            negb = tmp.tile([128, rc * C], f32, tag="negb")
            nc.vector.tensor_scalar_min(negb[:], zf, 0.0)
            eb = tmp.tile([128, rc * C], f32, tag="eb")
            nc.scalar.activation(eb[:], negb[:], AF.Exp)
            pmb = tmp.tile([128, rc * C], f32, tag="pmb")
            nc.vector.tensor_scalar(pmb[:], zf, 0.0, -1.0, op0=OP.max, op1=OP.add)
            zzb = outp.tile([128, rc, C], f32, tag="zzb", bufs=1)
            nc.vector.tensor_add(
                zzb.rearrange("p c m -> p (c m)"), eb[:], pmb[:]
            )
            # batched row-max (negated), then per-chunk exp/lse/final
            negm = stat.tile([128, rc], f32, tag="negm")
            nc.vector.tensor_reduce(
                negm[:], zzb[:], axis=mybir.AxisListType.X, op=OP.max, negate=True
            )
            ssum = stat.tile([128, rc], f32, tag="ssum")
            es = tmp.tile([128, rc * C], f32, tag="es")
            esv = es.rearrange("p (c m) -> p c m", m=C)
            for c in range(rc):
                nc.scalar.activation(
                    esv[:, c, :],
                    zzb[:, c, :],
                    AF.Exp,
                    bias=negm[:, c : c + 1],
                    accum_out=ssum[:, c : c + 1],
                )
            lse = stat.tile([128, rc], f32, tag="lse")
            nc.scalar.activation(lse[:], ssum[:], AF.Ln)
            for c in range(rc):
                osb = outp.tile([128, C], f32, tag="osb")
                nc.vector.tensor_scalar(
                    osb[:],
                    zzb[:, c, :],
                    negm[:, c : c + 1],
                    lse[:, c : c + 1],
                    op0=OP.add,
                    op1=OP.subtract,
                )
                ringC.dma_start(outr[c], osb[:])

    nc.compile()
    return nc


def make_in_maps(x, adj, W1, W2, Wout, bout, ncores=NCORES):
    n_total = adj.shape[0]
    s = n_total // ncores
    kb = n_total // 128
    kb8 = kb // 8
    f, d = W1.shape[1], W1.shape[0] * W1.shape[2]
    w1f = np.ascontiguousarray(
        W1.transpose(1, 0, 2).reshape(f, d).astype(np.float16)
    )
    w2f = np.ascontiguousarray(
        W2.transpose(1, 0, 2).reshape(d, d).astype(np.float16)
    )
    woutf = np.ascontiguousarray(Wout.astype(np.float32))
    bbf = np.ascontiguousarray(
        np.broadcast_to(bout.astype(np.float32), (128, Wout.shape[1]))
    )
    # xc[g*128 + p, ((j*2 + a)*128) + m] = x.T[a*128 + p, (g*8 + j)*128 + m]
    xt = x.T.astype(np.float16)  # [F, n_total]
    xcf = np.ascontiguousarray(
        xt.reshape(2, 128, kb8, 8, 128)
        .transpose(2, 1, 3, 0, 4)
        .reshape(kb8 * 128, 8 * f)
    )
    adj_t = adj.T  # view; [:, rows] below copies
    in_maps = []
    for c in range(ncores):
        rows = slice(c * s, (c + 1) * s)
        in_maps.append(
            {
                "adjt": adj_t[:, rows].astype(np.float16),
                "xc": xcf,
                "w1": w1f,
                "w2": w2f,
                "wout": woutf,
                "bb": bbf,
            }
        )
    return in_maps


def kernel(x, adj, W1, W2, Wout, bout):
    from concourse import bass_utils

    x = np.asarray(x)
    adj = np.asarray(adj)
    in_maps = make_in_maps(x, adj, np.asarray(W1), np.asarray(W2),
                           np.asarray(Wout), np.asarray(bout))
    if "nc" not in _nc_cache:
        _nc_cache["nc"] = build_gat_nc()
    res = bass_utils.run_bass_kernel_spmd(
        _nc_cache["nc"], in_maps, core_ids=list(range(NCORES))
    )
    return np.concatenate([r["out"] for r in res.results], axis=0).astype(np.float32)


# revision 19
# speedup vs baseline: 1.5228x; 1.0558x over previous
"""Bass/Trainium2 kernel for the (dead-attention) GAT reference.

Effective math (see reference):
    h1  = x @ W1f                 W1f = W1.transpose(1,0,2).reshape(256,128)
    hp1 = elu(adj @ h1)
    h2  = hp1 @ W2f               W2f = W2.transpose(1,0,2).reshape(128,128)
    hp2 = elu(adj @ h2)
    y   = elu(hp2 @ Wout + bout)
    out = log_softmax(y, axis=1)

Distribution: adj is sharded row-wise across 8 cores (2048 rows each),
uploaded pre-transposed + fp16, with the CONTRACTION rows rotated per
core so each core's own nodes come first.  h1 is computed REPLICATED on
every core (x is tiny), so layer 1 needs no collective.  h2 is
exchanged with one fp16 AllGather in feature-major layout; thanks to
the rotation each core starts layer 2 on its own h2 shard (local, no
collective wait) while the AllGather flies, then pulls the other 7
blocks with partition-id-indexed dynamic DMAs and transposes them back
to node-major lhsT tiles on the PE.  Each core streams its 67 MB adj
shard from HBM through the PE array twice:
    hpT[128 feat, 2048 rows] = sum_k h[kblk 128 rows].T-stationary @ adjT[kblk]
fp32 accumulation in PSUM; fp16 on the streamed matmuls (max elementwise
rel err vs the fp32 reference ~6e-4).
"""

import sys

import numpy as np

sys.path.insert(0, "/opt/trn_rl_repo")

N = 16384  # nodes
F = 256  # input features
D = 128  # hidden width (nheads*nhid)
C = 32  # classes
NCORES = 8
S = N // NCORES  # rows per core

_nc_cache = {}


def build_gat_nc(n_total=N, ncores=NCORES, enable_asserts=False, adj_bufs=6, kg=4):
    """Build the SPMD Bass program (one program, runs on all cores)."""
    from concourse import bacc, bass, masks, mybir, tile

    s = n_total // ncores  # shard rows per core
    kb = n_total // 128  # contraction blocks for the big matmul
    kb8 = kb // 8  # x chunk groups
    rc = s // 128  # 128-row chunks in this core's shard
    f32 = mybir.dt.float32
    f16 = mybir.dt.float16
    AF = mybir.ActivationFunctionType
    OP = mybir.AluOpType
    # n-chunks of the big-matmul output (<=512 fp32 per PSUM bank)
    nw = [min(512, s - i) for i in range(0, s, 512)]
    no = [i for i in range(0, s, 512)]

    nc = bacc.Bacc(
        "TRN2",
        target_bir_lowering=False,
        debug=False,
        enable_asserts=enable_asserts,
        num_devices=ncores,
    )

    adjt = nc.dram_tensor("adjt", [n_total, s], f16, kind="ExternalInput")
    xc = nc.dram_tensor("xc", [kb8 * 128, 8 * F], f16, kind="ExternalInput")
    w1 = nc.dram_tensor("w1", [F, D], f16, kind="ExternalInput")
    w2 = nc.dram_tensor("w2", [D, D], f16, kind="ExternalInput")
    wout = nc.dram_tensor("wout", [D, C], f32, kind="ExternalInput")
    bb = nc.dram_tensor("bb", [128, C], f32, kind="ExternalInput")
    # hoff[0, g] = ((rank + 1 + g) % ncores) * 128: gather-block row offsets
    hoff = nc.dram_tensor("hoff", [1, 8], mybir.dt.uint32, kind="ExternalInput")
    out = nc.dram_tensor("out", [s, C], f32, kind="ExternalOutput")

    rg = [list(range(ncores))]

    with tile.TileContext(nc) as tc:
        with (
            tc.tile_pool(name="dram", bufs=1, space="DRAM") as dram,
            tc.tile_pool(name="const", bufs=1) as const,
            tc.tile_pool(name="hfull", bufs=1) as hpool,
            tc.tile_pool(name="adjs", bufs=adj_bufs) as apool,
            tc.tile_pool(name="hblkp", bufs=4) as hblkp,
            tc.tile_pool(name="xcp", bufs=2) as xcpool,
            tc.tile_pool(name="xe", bufs=2) as xepool,
            tc.tile_pool(name="hsb", bufs=2) as hsbpool,
            tc.tile_pool(name="tmp", bufs=1) as tmp,
            tc.tile_pool(name="outp", bufs=2) as outp,
            tc.tile_pool(name="stat", bufs=1) as stat,
            tc.tile_pool(name="psb", bufs=4, space="PSUM") as psb,
            tc.tile_pool(name="pss", bufs=2, space="PSUM") as pss,
            tc.tile_pool(name="psy", bufs=2, space="PSUM") as psy,
        ):
            # two HWDGE rings (sync/scalar) alternate the big adj stream;
            # constants + tiny stores go to the SWDGE path (gpsimd)
            ringA, ringB, ringC = nc.sync, nc.scalar, nc.gpsimd

            # --- replicated constants (SWDGE so rings start streaming) ---
            w1s = const.tile([128, 2, D], f16, tag="w1s")
            ringC.dma_start(w1s[:], w1.ap().rearrange("(a p) m -> p a m", p=128))
            w2s = const.tile([128, D], f16, tag="w2s")
            ringC.dma_start(w2s[:], w2.ap())
            wouts = const.tile([128, C], f32, tag="wouts")
            ringC.dma_start(wouts[:], wout.ap())
            bbs = const.tile([128, C], f32, tag="bbs")
            ringC.dma_start(bbs[:], bb.ap())
            hoffs = const.tile([1, 8], mybir.dt.uint32, tag="hoffs")
            ringC.dma_start(hoffs[:], hoff.ap())
            ident = const.tile([128, 128], f16, tag="ident")
            masks.make_identity(nc, ident[:])

            # --- DRAM bounce buffers for the collective (feature-major) ---
            h2b = dram.tile([128, s], f16, tag="h2b")
            h2f = dram.tile([128 * ncores, s], f16, tag="h2f", addr_space="Shared")

            def big_layer(hs):
                # hpT[128 feat, s rows] += h[kblk].T-stationary @ adjT[kblk]
                ps = [
                    psb.tile([128, w], f32, tag="big", name=f"pbig{i}")
                    for i, w in enumerate(nw)
                ]
                ar = adjt.ap().rearrange("(g j p) m -> g p j m", j=kg, p=128)
                for g in range(kb // kg):
                    at = apool.tile([128, kg, s], f16, tag="adj")
                    (ringA if g % 2 == 0 else ringB).dma_start(at[:], ar[g])
                    for j in range(kg):
                        k = g * kg + j
                        for n, (o, w) in enumerate(zip(no, nw)):
                            nc.tensor.matmul(
                                ps[n][:],
                                hs[:, k, :],
                                at[:, j, o : o + w],
                                start=(k == 0),
                                stop=(k == kb - 1),
                            )
                return ps

            def elu_chunks(ps, dst):
                # dst[:, s] = elu(ps chunks), fp32
                for n, (o, w) in enumerate(zip(no, nw)):
                    neg = tmp.tile([128, 512], f32, tag="neg", name=f"neg{n}")
                    nc.vector.tensor_scalar_min(neg[:, :w], ps[n][:], 0.0)
                    ex = tmp.tile([128, 512], f32, tag="ex", name=f"ex{n}")
                    nc.scalar.activation(ex[:, :w], neg[:, :w], AF.Exp)
                    pm1 = tmp.tile([128, 512], f32, tag="pm1", name=f"pm1{n}")
                    nc.vector.tensor_scalar(
                        pm1[:, :w], ps[n][:], 0.0, -1.0, op0=OP.max, op1=OP.add
                    )
                    nc.vector.tensor_add(dst[:, o : o + w], ex[:, :w], pm1[:, :w])

            # ---- layer 1: h1 replicated (no collective) ----
            # xc group g holds 8 chunk-lhsTs contiguous per partition:
            # xc[g*128+p, ((j*2+a)*128)+m] = xrot.T[a*128+p, (g*8+j)*128+m]
            hs1 = hpool.tile([128, kb, D], f16, tag="hfull")
            xr = xc.ap().rearrange("(g p) q -> g p q", p=128)
            xg = None
            for k in range(kb):
                g, j = divmod(k, 8)
                if j == 0:
                    xg = xcpool.tile([128, 8, 2, 128], f16, tag="xg")
                    (ringA if g % 2 == 0 else ringB).dma_start(
                        xg.rearrange("p j a m -> p (j a m)"), xr[g]
                    )
                ph = pss.tile([128, D], f32, tag="pss", name=f"ph1_{k}")
                nc.tensor.matmul(
                    ph[:], xg[:, j, 0, :], w1s[:, 0, :], start=True, stop=False
                )
                nc.tensor.matmul(
                    ph[:], xg[:, j, 1, :], w1s[:, 1, :], start=False, stop=True
                )
                nc.vector.tensor_copy(hs1[:, k, :], ph[:])
            ps1 = big_layer(hs1)
            x2t = xepool.tile([128, s], f32, tag="xe")
            elu_chunks(ps1, x2t)

            # ---- layer 2 ----
            # own h2 shard (feature-major), start collective, and immediately
            # transpose the local shard into the first rc lhsT chunks
            x2h = xepool.tile([128, s], f16, tag="xeh")
            nc.vector.tensor_copy(x2h[:], x2t[:])
            h2sT = xepool.tile([128, s], f16, tag="h2sT")
            for c in range(rc):
                cs = slice(c * 128, (c + 1) * 128)
                ph2 = pss.tile([128, D], f32, tag="pss", name=f"ph2_{c}")
                # feat-major h2 chunk: W2f.T-stationary @ x2[feat, nodes]
                nc.tensor.matmul(ph2[:], w2s[:], x2h[:, cs], start=True, stop=True)
                nc.vector.tensor_copy(h2sT[:, cs], ph2[:])
            ringC.dma_start(h2b[:], h2sT[:])
            nc.gpsimd.collective_compute(
                "AllGather",
                OP.bypass,
                ins=[h2b.opt()],
                outs=[h2f.opt()],
                replica_groups=rg,
            )
            hs2 = hpool.tile([128, kb, D], f16, tag="hfull")
            for k in range(rc):  # own block: no collective wait
                pt = pss.tile([128, D], f16, tag="pss", name=f"ptl_{k}")
                nc.tensor.transpose(
                    pt[:], h2sT[:, k * 128 : (k + 1) * 128], ident[:]
                )
                nc.vector.tensor_copy(hs2[:, k, :], pt[:])
            # other ranks' blocks: dynamic row offset ((me+1+g) % ncores)*128
            # NOTE: keep these off the sync ring — SP-engine DMAs touching
            # collective-output buffers can hang (test_sync_dma_collective_hang)
            for g in range(ncores - 1):
                with ringB.register(f"hoffr{g}") as hreg:
                    ringB.reg_load(hreg, hoffs[0:1, g : g + 1])
                    off = ringB.snap(hreg, min_val=0, max_val=(ncores - 1) * 128)
                hb = hblkp.tile([128, s], f16, tag="hblk", name=f"hblk{g}")
                ringB.dma_start(hb[:], h2f[bass.ds(off, 128), :])
                for jj in range(rc):
                    k = rc * (1 + g) + jj
                    pt = pss.tile([128, D], f16, tag="pss", name=f"pt_{k}")
                    nc.tensor.transpose(
                        pt[:], hb[:, jj * 128 : (jj + 1) * 128], ident[:]
                    )
                    nc.vector.tensor_copy(hs2[:, k, :], pt[:])
            ps2 = big_layer(hs2)
            x3t = xepool.tile([128, s], f32, tag="xe")
            elu_chunks(ps2, x3t)

            # ---- output layer + log_softmax ----
            outr = out.ap().rearrange("(c p) m -> c p m", p=128)
            zbig = outp.tile([128, rc, C], f32, tag="zbig", bufs=1)
            for c in range(rc):
                py = psy.tile([128, C], f32, tag="psy")
                cs = slice(c * 128, (c + 1) * 128)
                nc.tensor.matmul(py[:], x3t[:, cs], wouts[:], start=True, stop=True)
                nc.vector.tensor_add(zbig[:, c, :], py[:], bbs[:])
            # batched elu over [128, rc*C]
            zf = zbig.rearrange("p c m -> p (c m)")
            negb = tmp.tile([128, rc * C], f32, tag="neg", name="negb")
            nc.vector.tensor_scalar_min(negb[:], zf, 0.0)
            eb = tmp.tile([128, rc * C], f32, tag="ex", name="eb")
            nc.scalar.activation(eb[:], negb[:], AF.Exp)
            pmb = tmp.tile([128, rc * C], f32, tag="pm1", name="pmb")
            nc.vector.tensor_scalar(pmb[:], zf, 0.0, -1.0, op0=OP.max, op1=OP.add)
            zzb = outp.tile([128, rc, C], f32, tag="zzb", bufs=1)
            nc.vector.tensor_add(
                zzb.rearrange("p c m -> p (c m)"), eb[:], pmb[:]
            )
            # batched row-max (negated), then per-chunk exp/lse/final
            negm = stat.tile([128, rc], f32, tag="negm")
            nc.vector.tensor_reduce(
                negm[:], zzb[:], axis=mybir.AxisListType.X, op=OP.max, negate=True
            )
            ssum = stat.tile([128, rc], f32, tag="ssum")
            es = tmp.tile([128, rc * C], f32, tag="neg", name="es")
            esv = es.rearrange("p (c m) -> p c m", m=C)
            for c in range(rc):
                nc.scalar.activation(
                    esv[:, c, :],
                    zzb[:, c, :],
                    AF.Exp,
                    bias=negm[:, c : c + 1],
                    accum_out=ssum[:, c : c + 1],
                )
            lse = stat.tile([128, rc], f32, tag="lse")
            nc.scalar.activation(lse[:], ssum[:], AF.Ln)
            for c in range(rc):
                osb = outp.tile([128, C], f32, tag="osb")
                nc.vector.tensor_scalar(
                    osb[:],
                    zzb[:, c, :],
                    negm[:, c : c + 1],
                    lse[:, c : c + 1],
                    op0=OP.add,
                    op1=OP.subtract,
                )
                ringC.dma_start(outr[c], osb[:])

    nc.compile()
    return nc


def make_in_maps(x, adj, W1, W2, Wout, bout, ncores=NCORES):
    n_total = adj.shape[0]
    s = n_total // ncores
    kb = n_total // 128
    kb8 = kb // 8
    f, d = W1.shape[1], W1.shape[0] * W1.shape[2]
    w1f = np.ascontiguousarray(
        W1.transpose(1, 0, 2).reshape(f, d).astype(np.float16)
    )
    w2f = np.ascontiguousarray(
        W2.transpose(1, 0, 2).reshape(d, d).astype(np.float16)
    )
    woutf = np.ascontiguousarray(Wout.astype(np.float32))
    bbf = np.ascontiguousarray(
        np.broadcast_to(bout.astype(np.float32), (128, Wout.shape[1]))
    )
    adj16 = adj.astype(np.float16)
    x16 = x.astype(np.float16)
    in_maps = []
    for c in range(ncores):
        rows = slice(c * s, (c + 1) * s)
        # rotate contraction rows so this core's own nodes come first
        rot = np.roll(np.arange(n_total), -c * s)
        adjtc = np.ascontiguousarray(adj16[rows][:, rot].T)
        # xc[g*128 + p, ((j*2 + a)*128) + m] = xrot.T[a*128 + p, (g*8 + j)*128 + m]
        xtc = x16[rot].T  # [F, n_total]
        xcf = np.ascontiguousarray(
            xtc.reshape(2, 128, kb8, 8, 128)
            .transpose(2, 1, 3, 0, 4)
            .reshape(kb8 * 128, 8 * f)
        )
        hoffc = np.zeros((1, 8), np.uint32)
        for g in range(ncores - 1):
            hoffc[0, g] = ((c + 1 + g) % ncores) * 128
        in_maps.append(
            {
                "adjt": adjtc,
                "xc": xcf,
                "w1": w1f,
                "w2": w2f,
                "wout": woutf,
                "bb": bbf,
                "hoff": hoffc,
            }
        )
    return in_maps


def kernel(x, adj, W1, W2, Wout, bout):
    from concourse import bass_utils

    x = np.asarray(x)
    adj = np.asarray(adj)
    in_maps = make_in_maps(x, adj, np.asarray(W1), np.asarray(W2),
                           np.asarray(Wout), np.asarray(bout))
    if "nc" not in _nc_cache:
        _nc_cache["nc"] = build_gat_nc()
    res = bass_utils.run_bass_kernel_spmd(
        _nc_cache["nc"], in_maps, core_ids=list(range(NCORES))
    )
    return np.concatenate([r["out"] for r in res.results], axis=0).astype(np.float32)
